# revision 4
# baseline (speedup 1.0000x reference)
"""CPAB transformer kernel for Trainium2 (8 NeuronCores, SPMD).

Same DVE-only knot-expansion as the baseline (which sits at the DVE
information bound of 32 two-term ops/step; cross-engine offload loses:
DVE and GPSIMD share SBUF ports, so Pool work slows DVE ~60%).

Change vs baseline: points are processed in two half-tiles so the
input DMA of half B and the output DMA of half A overlap compute,
hiding the ~100us serial DMA head/tail.  Out-DMAs are triggered inside
the per-theta branch (partition id materialized on both DVE and Pool).
"""

import numpy as np

NC = 32
NSTEPS = 32
N_THETA = 8
N_POINTS = 262144
P = 128
F = N_POINTS // P  # 2048
H = F // 2         # half tile

_KNOT_OP = None
_PROGRAM = None


def _register_dve_op():
    global _KNOT_OP
    if _KNOT_OP is not None:
        return _KNOT_OP
    import concourse.dve_ops as dve_ops
    from concourse.dve_ops import DveOp
    from concourse.dve_spec import Spec, Src0, Src1, C0, C1, C2, Zero, relu, select
    from concourse.dve_spec import lower as dve_lower
    from concourse.dve_uop import DveOpSpec

    for op in dve_ops.OPS:
        if op.name == "CPAB_KNOT":
            _KNOT_OP = op
            return op

    def _ref(in0, in1, s0, s1, imm2):
        x = in0.astype(np.float32)
        r = np.maximum(x - np.float32(imm2), 0).astype(np.float32)
        m1 = (r * np.float32(s0)).astype(np.float32)
        m2 = np.where(x >= np.float32(imm2), np.float32(s1), np.float32(0.0))
        return ((in1.astype(np.float32) + m1).astype(np.float32) + m2).astype(
            np.float32
        )

    body = Src1 + relu(Src0 - C2) * C0 + select(Src0 >= C2, C1, Zero)
    spec = Spec(body=body, reference=_ref)
    row = dve_ops._CUSTOM_DVE_ROW_BASE + len(dve_ops.OPS)
    shas = {}
    for ver in ("v3", "v4"):
        dspec = DveOpSpec(
            name="CPAB_KNOT", opcode=row, uops=dve_lower(spec, ver=ver), rd1_en=True
        )
        shas[ver] = dspec.sha(ver)
    op = DveOp("CPAB_KNOT", spec, subdim=False, uops_sha=shas)
    dve_ops.OPS.append(op)
    dve_ops.CUSTOM_DVE_SPECS[op.name] = op.spec
    dve_ops._SUB_OPCODE_FOR_NAME[op.name] = row
    _KNOT_OP = op
    return op


def _build_program(consts):
    global _PROGRAM
    key = consts.tobytes()
    if _PROGRAM is not None and _PROGRAM[0] == key:
        return _PROGRAM[1]
    import concourse.bacc as bacc
    import concourse.mybir as mybir
    from concourse.tile import TileContext

    knot = _register_dve_op()

    f32 = mybir.dt.float32
    nc = bacc.Bacc(
        "TRN2",
        target_bir_lowering=False,
        debug=False,
        num_devices=8,
        enable_partition_id=True,
    )
    pts = nc.dram_tensor("points", [P, F], f32, kind="ExternalInput").ap()
    out = nc.dram_tensor("out", [P, F], f32, kind="ExternalOutput").ap()

    mult = mybir.AluOpType.mult
    add = mybir.AluOpType.add
    E = mybir.EngineType

    with TileContext(nc) as tc:
        with tc.tile_pool(name="state", bufs=1) as pool:
            xs = [pool.tile([P, H], f32, name=f"x{h}", tag=f"x{h}")
                  for h in range(2)]
            ys = [pool.tile([P, H], f32, name=f"y{h}", tag=f"y{h}")
                  for h in range(2)]
            for h in range(2):
                nc.gpsimd.dma_start(xs[h][:], pts[:, h * H:(h + 1) * H])
            pid = nc.partition_id(engines=(E.DVE, E.Pool, E.Activation))
            for t in range(N_THETA):
                with tc.If(pid == t):
                    c = consts[t]
                    # interleave the two independent half-chains op-by-op:
                    # consecutive DVE instructions touch different buffers,
                    # hiding the write-ack latency of the in-place chain.
                    bufs = [(xs[0], ys[0]), (xs[1], ys[1])]
                    for _step in range(NSTEPS):
                        for h in range(2):
                            cur, nxt = bufs[h]
                            nc.scalar.activation(
                                nxt[:], cur[:],
                                mybir.ActivationFunctionType.Copy,
                                bias=float(c[63]), scale=float(c[62]),
                            )
                        for k in range(1, NC):
                            for h in range(2):
                                cur, nxt = bufs[h]
                                nc.vector._custom_dve(
                                    knot,
                                    out=nxt[:],
                                    in0=cur[:],
                                    in1=nxt[:],
                                    s0=float(c[k - 1]),
                                    s1=float(c[30 + k]),
                                    imm2=float(k) / NC,
                                )
                        bufs = [(n, c2) for (c2, n) in bufs]
                    # NSTEPS even: final state back in xs[h]
                    for h in range(2):
                        nc.gpsimd.dma_start(
                            out[:, h * H:(h + 1) * H], xs[h][:])
    nc.compile()
    _PROGRAM = (key, nc)
    return nc


def _host_tables(theta, basis):
    dT = 1.0 / NSTEPS
    Avees = basis.astype(np.float64) @ theta.astype(np.float64).T
    As = Avees.T.reshape(theta.shape[0] * NC, 2)
    a = dT * As[:, 0]
    b = dT * As[:, 1]
    small = np.abs(a) < 1e-6
    a_safe = np.where(small, 1.0, a)
    phi = np.where(small, 1.0 + 0.5 * a, np.expm1(a_safe) / a_safe)
    A = np.exp(a).reshape(theta.shape[0], NC)
    B = (b * phi).reshape(theta.shape[0], NC)
    return A, B


def _knot_consts(A, B):
    n_theta = A.shape[0]
    t_knots = np.arange(1, NC, dtype=np.float64) / NC
    gam = A[:, 1:] - A[:, :-1]
    dlt = (B[:, 1:] - B[:, :-1]) + gam * t_knots[None, :]
    consts = np.zeros((n_theta, 64), dtype=np.float32)
    consts[:, 0:31] = gam.astype(np.float32)
    consts[:, 31:62] = dlt.astype(np.float32)
    consts[:, 62] = A[:, 0].astype(np.float32)
    consts[:, 63] = B[:, 0].astype(np.float32)
    return consts


def kernel(points, theta, basis):
    from concourse.bass_utils import run_bass_kernel_spmd

    points = np.asarray(points)
    theta = np.asarray(theta)
    basis = np.asarray(basis)
    n_theta = theta.shape[0]
    assert points.shape == (1, N_POINTS) and n_theta == N_THETA

    A, B = _host_tables(theta, basis)
    consts = _knot_consts(A, B)
    pts_tile = np.ascontiguousarray(
        points[0].astype(np.float32).reshape(P, F)
    )

    nc = _build_program(consts)
    in_maps = [{"points": pts_tile} for _ in range(n_theta)]
    res = run_bass_kernel_spmd(nc, in_maps, list(range(n_theta)))
    out = np.stack(
        [res.results[t]["out"].reshape(N_POINTS) for t in range(n_theta)]
    )
    return out[:, None, :].astype(np.float32)
